# revision 1
# baseline (speedup 1.0000x reference)
"""Trainium2 Bass kernel for nn_AudioVisualSpikformer — v2.

Data-parallel over B=8 (core b gets batch b). Restructured vs v1:
 - k/v convs FIRST; their BN-stats AllReduce hides under the q conv.
 - q conv runs ONCE (h_q kept in SBUF, partly recycling h_k slots);
   the AR_q latency hides under interleaved attention kv matmuls.
 - h_v spills to DRAM during the k/v pass and is read back early.
 - kv matmul computes only the 128 needed output columns per half.
 - proj BN h_p recycles freed h_k/h_q slots; all convs 3-pass fp16
   split-precision (proj 2-pass: its input is exact in fp16).
"""
import sys
sys.path.insert(0, '/opt/trn_rl_repo')
import numpy as np

T, B, C, N, H = 4, 8, 256, 2048, 16
EPS = 1e-5
NCORES = 8
P = 128
KC = 2        # c_in chunks of 128
MH = 2        # c_out halves of 128
NT = 512      # matmul moving chunk
NW = 512      # psum group width (1 bank)
NG = N // NW  # 4 psum groups per (t, mh)
COUNT = T * B * N  # global BN count = 65536

_prog_cache = {}


def _build():
    import concourse.bacc as bacc
    import concourse.mybir as mybir
    from concourse import tile

    F32 = mybir.dt.float32
    FP16 = mybir.dt.float16
    FP8 = mybir.dt.float8e4
    AF = mybir.ActivationFunctionType
    ALU = mybir.AluOpType
    AX = mybir.AxisListType

    nc = bacc.Bacc("TRN2", target_bir_lowering=False, debug=False,
                   num_devices=NCORES, num_swdge_queues=4)

    xh_in = nc.dram_tensor("xh_in", [T * KC, P, N], FP16, kind="ExternalInput")
    xl_in = nc.dram_tensor("xl_in", [T * KC, P, N], FP16, kind="ExternalInput")
    yh_in = nc.dram_tensor("yh_in", [T * KC, P, N], FP16, kind="ExternalInput")
    yl_in = nc.dram_tensor("yl_in", [T * KC, P, N], FP16, kind="ExternalInput")
    wt_in = nc.dram_tensor("wt_in", [4, 2, P, KC * MH * P], FP16,
                           kind="ExternalInput")
    kvec_in = nc.dram_tensor("kvec_in", [P, 8], F32, kind="ExternalInput")
    mask_in = nc.dram_tensor("mask_in", [P, P], FP16,
                         kind="ExternalInput")
    out_d = nc.dram_tensor("out", [T * MH, P, N], FP8,
                       kind="ExternalOutput")

    with tile.TileContext(nc) as tc:
        with (
            tc.tile_pool(name="const", bufs=1) as cpool,
            tc.tile_pool(name="big", bufs=1) as bigp,
            tc.tile_pool(name="io", bufs=5) as iop,
            tc.tile_pool(name="spk", bufs=1) as spkp,
            tc.tile_pool(name="work", bufs=2) as wkp,
            tc.tile_pool(name="stat", bufs=1) as stp,
            tc.tile_pool(name="convps", bufs=5, space="PSUM") as convps,
            tc.tile_pool(name="kvps", bufs=1, space="PSUM") as kvps,
            tc.tile_pool(name="ops", bufs=2, space="PSUM") as ops,
            tc.tile_pool(name="dram", bufs=1, space="DRAM") as dramp,
        ):
            # ---------------- constants ----------------
            wt = cpool.tile([P, 8 * KC * MH * P], FP16, tag="wt")
            CW = KC * MH * P
            nc.sync.dma_start(out=wt[:, 2 * CW:3 * CW], in_=wt_in[1, 0, :, :])
            nc.sync.dma_start(out=wt[:, 3 * CW:4 * CW], in_=wt_in[1, 1, :, :])

            def wslice(j, lo, kc, mh):
                off = (j * 2 + lo) * (KC * MH * P) + (kc * MH + mh) * P
                return wt[:, off:off + P]

            kvec = cpool.tile([P, 8], F32, tag="kvec")
            nc.sync.dma_start(out=kvec[:], in_=kvec_in[:, :])
            mask = cpool.tile([P, P], FP16, tag="mask")
            nc.sync.dma_start(out=mask[:], in_=mask_in[:, :])
            attn_bias = cpool.tile([P, 1], F32, tag="attn_bias")
            nc.vector.memset(attn_bias[:], -1.5e30)

            sump = {(j, mh): stp.tile([P, 16], F32, tag=f"sump_{j}_{mh}",
                                      name=f"sump_{j}_{mh}")
                    for j in range(4) for mh in range(MH)}
            sqp = {(j, mh): stp.tile([P, 16], F32, tag=f"sqp_{j}_{mh}",
                                     name=f"sqp_{j}_{mh}")
                   for j in range(4) for mh in range(MH)}

            hqd = {(t, mh): dramp.tile([P, N], F32, tag=f"hqd_{t}_{mh}",
                                       name=f"hqd_{t}_{mh}")
                   for t in range(T) for mh in range(MH)}

            def conv_group(j, mh, hi_tiles, lo_tiles, dst_ap, col,
                           nm="cg"):
                psum = convps.tile([P, NW], F32, tag="convps", name=nm)
                po = psum[:]
                passes = []
                for kc in range(KC):
                    passes.append((wslice(j, 0, kc, mh), hi_tiles[kc]))
                    if lo_tiles is not None:
                        passes.append((wslice(j, 0, kc, mh), lo_tiles[kc]))
                    passes.append((wslice(j, 1, kc, mh), hi_tiles[kc]))
                for i, (w_ap, m_ap) in enumerate(passes):
                    nc.tensor.matmul(po, w_ap, m_ap, start=(i == 0),
                                     stop=(i == len(passes) - 1))
                nc.scalar.activation(
                    out=dst_ap, in_=psum[:], func=AF.Copy,
                    accum_out=sump[(j, mh)][:, col:col + 1])
                scr = wkp.tile([P, NW], F32, tag="ostage", bufs=1,
                               name=f"scr_{nm}")
                nc.vector.scalar_tensor_tensor(
                    out=scr[:], in0=dst_ap, scalar=1.0, in1=dst_ap,
                    op0=ALU.mult, op1=ALU.mult,
                    accum_out=sqp[(j, mh)][:, col:col + 1])

            def thr_math(gs, ncols, kvec_ap, tag):
                inv = 1.0 / COUNT
                mean = stp.tile([P, ncols], F32, tag=f"mean_{tag}",
                                name=f"mean_{tag}")
                nc.vector.tensor_scalar(out=mean[:], in0=gs[:, 0:ncols],
                                        scalar1=inv, scalar2=None, op0=ALU.mult)
                ex2 = stp.tile([P, ncols], F32, tag=f"ex2_{tag}",
                               name=f"ex2_{tag}")
                nc.vector.tensor_scalar(out=ex2[:], in0=gs[:, ncols:2 * ncols],
                                        scalar1=inv, scalar2=None, op0=ALU.mult)
                var = stp.tile([P, ncols], F32, tag=f"var_{tag}",
                               name=f"var_{tag}")
                m2 = stp.tile([P, ncols], F32, tag=f"m2_{tag}", name=f"m2_{tag}")
                nc.vector.tensor_tensor(out=m2[:], in0=mean[:], in1=mean[:],
                                        op=ALU.mult)
                nc.vector.tensor_tensor(out=var[:], in0=ex2[:], in1=m2[:],
                                        op=ALU.subtract)
                nc.vector.tensor_scalar(out=var[:], in0=var[:], scalar1=EPS,
                                        scalar2=None, op0=ALU.add)
                s0 = stp.tile([P, ncols], F32, tag=f"s0_{tag}", name=f"s0_{tag}")
                nc.scalar.activation(out=s0[:], in_=var[:], func=AF.Sqrt)
                r0 = stp.tile([P, ncols], F32, tag=f"r0_{tag}", name=f"r0_{tag}")
                nc.vector.reciprocal(out=r0[:], in_=s0[:])
                s1 = stp.tile([P, ncols], F32, tag=f"s1_{tag}", name=f"s1_{tag}")
                nc.vector.tensor_tensor(out=s1[:], in0=var[:], in1=r0[:],
                                        op=ALU.mult)
                nc.vector.tensor_tensor(out=s1[:], in0=s1[:], in1=s0[:],
                                        op=ALU.add)
                nc.vector.tensor_scalar(out=s1[:], in0=s1[:], scalar1=0.5,
                                        scalar2=None, op0=ALU.mult)
                ks = stp.tile([P, ncols], F32, tag=f"ks_t_{tag}",
                              name=f"ks_t_{tag}")
                nc.vector.tensor_tensor(out=ks[:], in0=kvec_ap, in1=s1[:],
                                        op=ALU.mult)
                thr = stp.tile([P, ncols], F32, tag=f"thr_{tag}",
                               name=f"thr_{tag}")
                nc.vector.tensor_tensor(out=thr[:], in0=mean[:], in1=ks[:],
                                        op=ALU.add)
                return thr

            def load2(dram_h, dram_l, t, kc, ng):
                sl = (t * KC + kc, slice(None), slice(ng * NT, (ng + 1) * NT))
                a = iop.tile([P, NT], FP16, tag="mvh", bufs=5, name="mvh_t")
                nc.sync.dma_start(out=a[:], in_=dram_h[sl[0], sl[1], sl[2]])
                b = iop.tile([P, NT], FP16, tag="mvl", bufs=5, name="mvl_t")
                nc.sync.dma_start(out=b[:], in_=dram_l[sl[0], sl[1], sl[2]])
                return a, b

            # ============ Phase A: k + v convs (+stats) ============
            hK = {}
            hV = {}
            wt_stage = [0]

            def emit_wt_rest():
                if wt_stage[0] == 0:
                    nc.sync.dma_start(out=wt[:, 4 * CW:5 * CW],
                                      in_=wt_in[2, 0, :, :])
                    nc.sync.dma_start(out=wt[:, 5 * CW:6 * CW],
                                      in_=wt_in[2, 1, :, :])
                elif wt_stage[0] == 1:
                    nc.sync.dma_start(
                        out=wt[:, 0:2 * CW],
                        in_=wt_in.rearrange("j l p c -> p (j l) c")[:, 0:2, :])
                    nc.sync.dma_start(
                        out=wt[:, 6 * CW:],
                        in_=wt_in.rearrange("j l p c -> p (j l) c")[:, 6:, :])
                wt_stage[0] += 1

            for t in range(T):
                for mh in range(MH):
                    hK[(t, mh)] = bigp.tile([P, N], F32, tag=f"hk_{t}_{mh}",
                                            name=f"hk_{t}_{mh}")
                    hV[(t, mh)] = bigp.tile([P, N], F32, tag=f"hv_{t}_{mh}",
                                            name=f"hv_{t}_{mh}")
                for ng in range(NG):
                    yh = {}
                    yl = {}
                    for kc in range(KC):
                        a, b = load2(yh_in, yl_in, t, kc, ng)
                        yh[kc] = a[:]
                        yl[kc] = b[:]
                    if t == 0 and ng < 2:
                        emit_wt_rest()
                    for mh in range(MH):
                        conv_group(1, mh, yh, yl,
                                   hK[(t, mh)][:, ng * NW:(ng + 1) * NW],
                                   t * NG + ng, nm=f"kps_{t}_{mh}_{ng}")
                    for mh in range(MH):
                        conv_group(2, mh, yh, yl,
                                   hV[(t, mh)][:, ng * NW:(ng + 1) * NW],
                                   t * NG + ng, nm=f"vps_{t}_{mh}_{ng}")
                tc.no_sync_barrier()

            # kv stats -> AllReduce (hides under phase B)
            statsKV = stp.tile([P, 8], F32, tag="statsKV")
            for j in (1, 2):
                for mh in range(MH):
                    c = (j - 1) * 2 + mh
                    nc.vector.tensor_reduce(
                        out=statsKV[:, c:c + 1], in_=sump[(j, mh)][:],
                        axis=AX.X, op=ALU.add)
                    nc.vector.tensor_reduce(
                        out=statsKV[:, 4 + c:5 + c], in_=sqp[(j, mh)][:],
                        axis=AX.X, op=ALU.add)
            def ag_issue(name, stats_ap, ncols):
                di = dramp.tile([P, ncols], F32, tag=f"agi_{name}",
                                name=f"agi_{name}")
                do = dramp.tile([NCORES * P, ncols], F32, tag=f"ago_{name}",
                                name=f"ago_{name}")
                nc.sync.dma_start(out=di[:], in_=stats_ap)
                nc.gpsimd.collective_compute(
                    "AllGather", ALU.bypass,
                    replica_groups=[list(range(NCORES))],
                    ins=[di[:].opt()], outs=[do[:].opt()])
                return do

            def ag_reduce(name, do, ncols, eng=None):
                g = stp.tile([P, NCORES * ncols], F32, tag=f"g_{name}",
                             name=f"g_{name}")
                (eng or nc.sync).dma_start(
                    out=g[:].rearrange("p (r c) -> p r c", r=NCORES),
                    in_=do[:].rearrange("(r p) c -> p r c", p=P))
                for half in (4, 2, 1):
                    nc.vector.tensor_tensor(
                        out=g[:, :half * ncols], in0=g[:, :half * ncols],
                        in1=g[:, half * ncols:2 * half * ncols], op=ALU.add)
                return g

            dkvo = ag_issue("kv", statsKV[:], 8)

            # ============ Phase B: q conv -> DRAM spill ============
            k_s = {}
            v_s = {}
            kT = {}
            vT = {}
            kvb = {}

            def emit_kv_spikes(t, thrKV, negthrV):
                for mh in range(MH):
                    ksx = spkp.tile([P, N], FP16, tag=f"ks_{mh}", bufs=2,
                                    name=f"ks_{t}_{mh}")
                    nc.vector.tensor_scalar(
                        out=ksx[:], in0=hK[(t, mh)][:],
                        scalar1=thrKV[:, 0 + mh:1 + mh],
                        scalar2=None, op0=ALU.is_ge)
                    k_s[(t, mh)] = ksx
                    vsx = spkp.tile([P, N], FP16, tag=f"vs_{mh}", bufs=1,
                                    name=f"vs_{t}_{mh}")
                    nc.scalar.activation(
                        out=vsx[:], in_=hV[(t, mh)][:], func=AF.Sigmoid,
                        scale=1e30, bias=negthrV[:, mh:mh + 1])
                    v_s[(t, mh)] = vsx

            def emit_transposes(t):
                kTt = spkp.tile([P, 16 * C], FP16, tag="kT", bufs=1,
                                name=f"kT_{t}")
                vTt = spkp.tile([P, 16 * C], FP16, tag="vT", bufs=1,
                                name=f"vT_{t}")
                for mh in range(MH):
                    nc.sync.dma_start_transpose(
                        out=kTt[:].rearrange("p (nn c) -> p nn c", c=C)
                            [:, :, mh * P:(mh + 1) * P],
                        in_=k_s[(t, mh)][:])
                    nc.sync.dma_start_transpose(
                        out=vTt[:].rearrange("p (nn c) -> p nn c", c=C)
                            [:, :, mh * P:(mh + 1) * P],
                        in_=v_s[(t, mh)][:])
                kT[t] = kTt
                vT[t] = vTt

            def emit_kv_matmul(t):
                kvbt = wkp.tile([P, C], FP16, tag="kvb", bufs=4,
                                name=f"kvb_{t}")
                for mh in range(MH):
                    pk = kvps.tile([P, P], F32, tag="kvps", bufs=1,
                                   name=f"kvps_{t}_{mh}")
                    for nn in range(16):
                        nc.tensor.matmul(
                            pk[:],
                            kT[t][:, nn * C + mh * P: nn * C + (mh + 1) * P],
                            vT[t][:, nn * C + mh * P: nn * C + (mh + 1) * P],
                            start=(nn == 0), stop=(nn == 15))
                    nc.vector.tensor_tensor(
                        out=kvbt[:, mh * P:(mh + 1) * P],
                        in0=pk[:], in1=mask[:], op=ALU.mult)
                kvb[t] = kvbt

            thrKV = None
            negthrV = None
            for t in range(T):
                for ng in range(NG):
                    xh = {}
                    xl = {}
                    for kc in range(KC):
                        a, b = load2(xh_in, xl_in, t, kc, ng)
                        xh[kc] = a[:]
                        xl[kc] = b[:]
                    for mh in range(MH):
                        cst = wkp.tile([P, NW], F32, tag="cstage", bufs=3,
                                       name=f"cst_{t}_{mh}_{ng}")
                        conv_group(0, mh, xh, xl, cst[:],
                                   t * NG + ng, nm=f"qps_{t}_{mh}_{ng}")
                        nc.gpsimd.dma_start(
                            out=hqd[(t, mh)][:, ng * NW:(ng + 1) * NW],
                            in_=cst[:])
                if t == 2:
                    # AR_kv done by now; compute global thresholds
                    gstatsKV = ag_reduce("kv", dkvo, 8)
                    thrKV = thr_math(gstatsKV, 4, kvec[:, 2:6], "kv")
                    negthrV = stp.tile([P, 2], F32, tag="negthrV")
                    nc.vector.tensor_scalar(
                        out=negthrV[:], in0=thrKV[:, 2:4],
                        scalar1=-1e30, scalar2=None, op0=ALU.mult)
                tc.no_sync_barrier()

            # q stats -> AllReduce issued ASAP
            statsQ = stp.tile([P, 4], F32, tag="statsQ")
            for mh in range(MH):
                nc.vector.tensor_reduce(
                    out=statsQ[:, mh:mh + 1], in_=sump[(0, mh)][:],
                    axis=AX.X, op=ALU.add)
                nc.vector.tensor_reduce(
                    out=statsQ[:, 2 + mh:3 + mh], in_=sqp[(0, mh)][:],
                    axis=AX.X, op=ALU.add)
            dqo = ag_issue("q", statsQ[:], 4)

            # spikes + transposes + kv matmuls (hide under AR_q);
            # h_q reads back into the freed h_k slots
            hQB = {}

            def emit_rb(t):
                for mh in range(MH):
                    hqb = bigp.tile([P, N], F32, tag=f"hk_{t}_{mh}",
                                    name=f"hqb_{t}_{mh}")
                    nc.sync.dma_start(out=hqb[:], in_=hqd[(t, mh)][:, :])
                    hQB[(t, mh)] = hqb

            for t in range(T):
                emit_kv_spikes(t, thrKV, negthrV)
            for t in range(T):
                emit_transposes(t)
                emit_kv_matmul(t)
            tc.no_sync_barrier()

            gstatsQ = ag_reduce("q", dqo, 4, eng=nc.gpsimd)
            thrQ = thr_math(gstatsQ, 2, kvec[:, 0:2], "q")

            # ============ Phase C: attention + proj ============
            hP = {}
            s01h = {}

            def emit_o_all(t):
                s01 = {}
                for mh in range(MH):
                    s01[mh] = spkp.tile([P, N], FP16, tag=f"ks_{mh}", bufs=2,
                                        name=f"s01_{t}_{mh}")
                    hp = bigp.tile([P, N], F32, tag=f"hv_{t}_{mh}",
                                   name=f"hp_{t}_{mh}")
                    hP[(t, mh)] = hp
                s01h[t] = s01
                for nch in range(NG):
                    for mh in range(MH):
                        qhb = wkp.tile([P, NT], F32, tag="qhb", bufs=3,
                                       name=f"qhb_{t}_{mh}_{nch}")
                        nc.sync.dma_start(
                            out=qhb[:],
                            in_=hqd[(t, mh)][:, nch * NT:(nch + 1) * NT])
                        qs = wkp.tile([P, NT], FP16, tag="qs", bufs=3,
                                      name=f"qs_{t}_{mh}_{nch}")
                        nc.vector.tensor_scalar(
                            out=qs[:], in0=qhb[:],
                            scalar1=thrQ[:, mh:mh + 1],
                            scalar2=None, op0=ALU.is_ge)
                        po = ops.tile([P, NT], F32, tag="ops",
                                      name=f"ops_{t}_{mh}_{nch}")
                        nc.tensor.matmul(
                            po[:], kvb[t][:, mh * P:(mh + 1) * P], qs[:],
                            start=True, stop=True)
                        if mh == 0:
                            nc.scalar.activation(
                                out=s01[mh][:, nch * NT:(nch + 1) * NT],
                                in_=po[:], func=AF.Sigmoid, scale=1e30,
                                bias=attn_bias[:])
                        else:
                            nc.vector.tensor_scalar(
                                out=s01[mh][:, nch * NT:(nch + 1) * NT],
                                in0=po[:], scalar1=1.5, scalar2=None,
                                op0=ALU.is_ge)

            def emit_proj_all(t):
                s01 = s01h[t]
                for nch in range(NG):
                    for mh in range(MH):
                        hi_tiles = {kc: s01[kc][:, nch * NT:(nch + 1) * NT]
                                    for kc in range(KC)}
                        conv_group(3, mh, hi_tiles, None,
                                   hP[(t, mh)][:, nch * NW:(nch + 1) * NW],
                                   t * NG + nch, nm=f"pps_{t}_{mh}_{nch}")

            emit_o_all(0)
            for t in range(1, T):
                emit_o_all(t)
                emit_proj_all(t - 1)
            emit_proj_all(T - 1)

            # proj stats -> AllReduce -> final threshold
            statsP = stp.tile([P, 4], F32, tag="statsP")
            for mh in range(MH):
                nc.vector.tensor_reduce(
                    out=statsP[:, mh:mh + 1], in_=sump[(3, mh)][:],
                    axis=AX.X, op=ALU.add)
                nc.vector.tensor_reduce(
                    out=statsP[:, 2 + mh:3 + mh], in_=sqp[(3, mh)][:],
                    axis=AX.X, op=ALU.add)
            d2o = ag_issue("p", statsP[:], 4)
            gstatsP = ag_reduce("p", d2o, 4)
            thrP = thr_math(gstatsP, 2, kvec[:, 6:8], "proj")
            negthrP = stp.tile([P, 2], F32, tag="negthrP")
            nc.vector.tensor_scalar(out=negthrP[:], in0=thrP[:],
                                    scalar1=-1e30, scalar2=None, op0=ALU.mult)

            for t in range(T):
                for mh in range(MH):
                    hp = hP[(t, mh)]
                    og = wkp.tile([P, N], FP8, tag="cstage", bufs=3,
                                  name=f"og_{t}_{mh}")
                    nc.vector.tensor_scalar(
                        out=og[:], in0=hp[:],
                        scalar1=thrP[:, mh:mh + 1], scalar2=None,
                        op0=ALU.is_ge)
                    nc.sync.dma_start(
                        out=out_d[t * MH + mh, :, :], in_=og[:])

    nc.finalize()
    return nc


def _get_prog():
    if "nc" not in _prog_cache:
        _prog_cache["nc"] = _build()
    return _prog_cache["nc"]


def _split16(a):
    hi = a.astype(np.float16)
    lo = (a - hi.astype(np.float32)).astype(np.float16)
    return hi, lo


def _prep_in_maps(x, y, q_w, q_gamma, q_beta, k_w, k_gamma, k_beta,
                  v_w, v_gamma, v_beta, proj_w, proj_gamma, proj_beta):
    x = np.asarray(x, dtype=np.float32)
    y = np.asarray(y, dtype=np.float32)

    def wt_host(w):
        w = np.asarray(w, dtype=np.float32)
        a = w.reshape(MH, P, KC, P)          # [mh, c, kc, p]
        lhsT = np.ascontiguousarray(a.transpose(3, 2, 0, 1).reshape(P, KC * MH * P))
        return _split16(lhsT)

    wts = np.empty((4, 2, P, KC * MH * P), dtype=np.float16)
    for j, w in enumerate([q_w, k_w, v_w, proj_w]):
        hi, lo = wt_host(w)
        wts[j, 0] = hi
        wts[j, 1] = lo

    def kvec_host(gamma, beta):
        g = np.asarray(gamma, dtype=np.float64)
        b = np.asarray(beta, dtype=np.float64)
        return ((1.0 - b) / g).astype(np.float32)

    kv6 = np.zeros((P, 8), dtype=np.float32)
    for j, (g, b) in enumerate([(q_gamma, q_beta), (k_gamma, k_beta),
                                (v_gamma, v_beta)]):
        kvj = kvec_host(g, b).reshape(MH, P)
        kv6[:, 2 * j + 0] = kvj[0]
        kv6[:, 2 * j + 1] = kvj[1]
    kvp = kvec_host(proj_gamma, proj_beta).reshape(MH, P)
    kv6[:, 6] = kvp[0]
    kv6[:, 7] = kvp[1]

    mask = np.zeros((P, P), dtype=np.float16)
    for h in range(P // 16):
        mask[h * 16:(h + 1) * 16, h * 16:(h + 1) * 16] = 1.0

    in_maps = []
    for b in range(NCORES):
        xb = np.ascontiguousarray(x[:, b].reshape(T * KC, P, N))
        yb = np.ascontiguousarray(y[:, b].reshape(T * KC, P, N))
        xhb, xlb = _split16(xb)
        yhb, ylb = _split16(yb)
        in_maps.append(dict(xh_in=xhb, xl_in=xlb, yh_in=yhb, yl_in=ylb,
                            wt_in=wts, kvec_in=kv6, mask_in=mask))
    return in_maps


def _assemble(res):
    out = np.empty((T, B, C, N), dtype=np.float32)
    for b in range(NCORES):
        ob = res.results[b]["out"]          # [T*MH, P, N] fp8 {0,1}
        out[:, b] = ob.reshape(T, C, N).astype(np.float32)
    return out


def kernel(**inputs):
    from concourse.bass_utils import run_bass_kernel_spmd
    in_maps = _prep_in_maps(**inputs)
    nc = _get_prog()
    res = run_bass_kernel_spmd(nc, in_maps, list(range(NCORES)))
    return _assemble(res)


def run_traced(**inputs):
    from concourse.bass_utils import run_bass_kernel_spmd
    in_maps = _prep_in_maps(**inputs)
    nc = _get_prog()
    res = run_bass_kernel_spmd(nc, in_maps, list(range(NCORES)), trace=True)
    res.out = _assemble(res)
    return res

